# revision 10
# baseline (speedup 1.0000x reference)
"""Trainium2 Bass kernel for nn_AdvOneLayer (dense_mlp, memory-bound).

Math (see the PyTorch/JAX reference):
    W1_norm[j] = sum_i |W1[j, i]|                       # [H]
    pert[b,i,j] = -eps * y[b,i] * sign(W2[i,j]) * W1_norm[j]   # [B, O, H]
    nn_output[b,i] = H * sum_j W2[i,j] + bias2[i]       # [B, O], independent of b

Sharding: H (=4096) is split 512-per-core across 8 NeuronCores.  Each core
reads only its W1/W2 slice (2MB + 0.5MB) plus the replicated y (64KB) and
writes its 32MB slab of pert.  The tiny nn_output reduction over j is done
as per-core partials that are summed on the host during the gather step
(the "all-reduce on the sum over j" from the sharding hint, realized at
unshard time).

Per-core dataflow:
  - W1 slice [512,1024] -> 4 SBUF tiles [128,1024]; DVE abs-reduce ->
    norm_col4[p,t] (partition-major norms).
  - PE transpose [128,4]->[4,128], then 4 rank-1 matmuls (ones x norm_row)
    broadcast the norms to all partitions: bnorm[q, j] = norm[j].
  - C[h] = sign(W2_h) * bnorm   for the two 128-row halves of O=256.
  - A[p, t] = -eps * yT[p, t]  where t indexes the 128 row-tiles of the
    flattened (b,i) dim: flat = t*128 + p, i = (t%2)*128 + p.
  - 128 output tiles: pert_tile[p, :] = A[p, t] * C[t%2][p, :], staged in
    4MB chunks (16 tiles) and written with one large DMA each.
"""

import sys

sys.path.insert(0, "/opt/trn_rl_repo")

import numpy as np

import concourse.bass as bass
import concourse.tile as tile
from concourse import masks, mybir
from concourse.bass_utils import run_bass_kernel_spmd
from concourse.vector_clock import ScopedClock


def _patched_drain_and_barrier(self, tick_clock, wait_clock):
    """Replacement for TileContext._drain_and_barrier: the walrus codegen in
    this toolchain allows only a limited number of sync waits per instruction,
    so spread the end-of-kernel drain's waits over consecutive single-wait
    drain instructions instead of attaching all of them to one."""
    drain_inst = self.nc.sync.drain()
    wait_clock.add_sem_waits(
        drain_inst.ins, ScopedClock({None: tick_clock.global_clock})
    )
    si = drain_inst.ins.sync_info
    if si is not None and si.on_wait and len(si.on_wait) > 1:
        waits = list(si.on_wait)
        si.on_wait = waits[:1]
        for w in waits[1:]:
            extra = self.nc.sync.drain()
            extra.ins.sync_info = mybir.SyncInfo(on_wait=[w], on_update=[])

    self.nc.all_engine_barrier()
    assert self.sems is not None
    popped = self.nc._tile_sem_poison_stack.pop()
    assert popped is self._sem_poison
    self.nc.clear_and_free_semaphores(list(self.sems.allocated().values()))
    self.nc.all_engine_barrier()


tile.TileContext._drain_and_barrier = _patched_drain_and_barrier

EPS = 0.1
B, I, O, H = 64, 1024, 256, 4096
NCORES = 8
HS = H // NCORES            # 512 hidden units per core
T = (B * O) // 128          # 128 row-tiles of the flattened (b,i) dim
G = 16                      # tiles per store chunk -> 16*128*512*4 = 4 MB DMA

_NC_CACHE = None


def _build_kernel():
    nc = bass.Bass()
    f32 = mybir.dt.float32
    w1s = nc.declare_dram_parameter("w1s", [HS, I], f32, isOutput=False)
    w2s = nc.declare_dram_parameter("w2s", [O, HS], f32, isOutput=False)
    yt = nc.declare_dram_parameter("yt", [128, T], f32, isOutput=False)
    pert_out = nc.declare_dram_parameter("pert_out", [T, 128, HS], f32, isOutput=True)
    s_out = nc.declare_dram_parameter("s_out", [128, 2], f32, isOutput=True)

    with tile.TileContext(nc, pool_alloc_mode="queue") as tc:
        with (
            tc.tile_pool(name="singles", bufs=1) as singles,
            tc.tile_pool(name="work", bufs=2) as work,
            tc.tile_pool(name="stage", bufs=2) as stagep,
            tc.tile_pool(name="psum", bufs=1, space="PSUM") as psum,
        ):
            # ---- load inputs -------------------------------------------------
            w1_sb = singles.tile([128, 4, I], f32)
            nc.sync.dma_start(out=w1_sb[:], in_=w1s.rearrange("(t p) i -> p t i", p=128))
            w2_sb = singles.tile([128, 2, HS], f32)
            nc.sync.dma_start(out=w2_sb[:], in_=w2s.rearrange("(h p) j -> p h j", p=128))
            yt_sb = singles.tile([128, T], f32)
            nc.sync.dma_start(out=yt_sb[:], in_=yt[:, :])

            # ---- W1 row 1-norms: norm_col4[p, t] = sum_i |W1s[t*128+p, i]| ---
            norm_col4 = singles.tile([128, 4], f32)
            for t4 in range(4):
                nc.vector.tensor_reduce(
                    out=norm_col4[:, t4 : t4 + 1],
                    in_=w1_sb[:, t4, :],
                    axis=mybir.AxisListType.X,
                    op=mybir.AluOpType.add,
                    apply_absolute_value=True,
                )

            # ---- norms to free-dim layout, broadcast across partitions -------
            identity = singles.tile([128, 128], f32)
            masks.make_identity(nc, identity[:])
            # PE instructions only get one sync-wait slot; bounce the norms
            # through gpsimd so the transpose waits on a single Pool semaphore
            # (covering both the identity build and the norm reduces).
            norm_col4g = singles.tile([128, 4], f32)
            nc.gpsimd.tensor_copy(norm_col4g[:], norm_col4[:])
            normT_ps = psum.tile([4, 128], f32)
            nc.tensor.transpose(normT_ps[:], norm_col4g[:], identity[:])
            normT_sb = singles.tile([4, 128], f32)
            nc.vector.tensor_copy(normT_sb[:], normT_ps[:])
            # bounce the norms through DRAM to get them replicated across all
            # 128 partitions in free-dim-major order: scratch[j] = norm[j]
            norm_scratch = nc.dram_tensor("norm_scratch", [1, HS], f32)
            nc.sync.dma_start(out=norm_scratch[:, :], in_=normT_sb[:])
            bnorm_sb = singles.tile([128, HS], f32)
            nc.sync.dma_start(
                out=bnorm_sb[:], in_=norm_scratch[0:1, :].partition_broadcast(128)
            )

            # ---- C[h] = sign(W2_h) * norm ------------------------------------
            # sign computed on DVE (is_gt - is_lt) so the whole C chain stays
            # on one engine; the walrus codegen allows only ONE semaphore wait
            # per compute instruction, so deps must funnel through one engine.
            c_sb = singles.tile([128, 2, HS], f32)
            for h in range(2):
                gt_t = work.tile([128, HS], f32)
                nc.vector.tensor_scalar(
                    out=gt_t[:], in0=w2_sb[:, h, :], scalar1=0.0, scalar2=None,
                    op0=mybir.AluOpType.is_gt,
                )
                lt_t = work.tile([128, HS], f32)
                nc.vector.tensor_scalar(
                    out=lt_t[:], in0=w2_sb[:, h, :], scalar1=0.0, scalar2=None,
                    op0=mybir.AluOpType.is_lt,
                )
                sgn_t = work.tile([128, HS], f32)
                nc.vector.tensor_sub(sgn_t[:], gt_t[:], lt_t[:])
                nc.vector.tensor_mul(c_sb[:, h, :], sgn_t[:], bnorm_sb[:])

            # ---- per-core partial row sums of W2 (for nn_output) -------------
            s_sb = singles.tile([128, 2], f32)
            for h in range(2):
                nc.vector.reduce_sum(
                    out=s_sb[:, h : h + 1],
                    in_=w2_sb[:, h, :],
                    axis=mybir.AxisListType.X,
                )
            nc.sync.dma_start(out=s_out[:, :], in_=s_sb[:])

            # ---- A[p, t] = -eps * yT[p, t] ----------------------------------
            a_sb = singles.tile([128, T], f32)
            nc.vector.tensor_scalar_mul(a_sb[:], yt_sb[:], -EPS)

            # ---- main loop: pert row-tiles, 4MB store chunks ----------------
            for chunk in range(T // G):
                stage = stagep.tile([128, G, HS], f32)
                for g in range(G):
                    t = chunk * G + g
                    nc.vector.tensor_scalar_mul(
                        out=stage[:, g, :],
                        in0=c_sb[:, t % 2, :],
                        scalar1=a_sb[:, t : t + 1],
                    )
                nc.sync.dma_start(
                    out=pert_out[chunk * G : (chunk + 1) * G].rearrange("t p j -> p t j"),
                    in_=stage[:],
                )
    return nc


def _strip_dead_same_engine_waits(nc):
    """Drop semaphore waits that are provably satisfied by same-engine program
    order.  The walrus codegen here only supports ONE sync wait per compute
    instruction, and Tile sometimes pairs a necessary cross-engine wait with a
    dead same-engine one (e.g. a DVE instruction waiting on the DVE-owned
    semaphore at a value already reached by its predecessors).

    A wait is dropped only if (a) the semaphore is updated exclusively by
    instructions on the waiting instruction's own engine, and (b) the
    cumulative increments from preceding same-engine instructions in program
    order already reach the wait value.  On an in-order engine (DVE drains
    after every op) such a wait can never block, so removing it is a no-op
    semantically.
    """
    from collections import defaultdict

    fn = nc.m.functions[0]
    upd_engines = defaultdict(set)
    for bb in fn.blocks:
        for inst in bb.instructions:
            si = getattr(inst, "sync_info", None)
            if si:
                for u in si.on_update or []:
                    upd_engines[u.id].add(str(inst.engine))

    cum = defaultdict(int)  # (engine, sem_id) -> total increments so far
    n_drop = 0
    for bb in fn.blocks:
        for inst in bb.instructions:
            si = getattr(inst, "sync_info", None)
            if si is None:
                continue
            eng = str(inst.engine)
            if si.on_wait and len(si.on_wait) > 1:
                keep = []
                for w in si.on_wait:
                    dead = (
                        w.sync_type == "semaphore"
                        and str(w.wait_mode) in ("sem-ge-imm", "WaitMode.sem_ge_imm")
                        and upd_engines.get(w.id) == {eng}
                        and cum[(eng, w.id)] >= (w.wait_value or 0)
                    )
                    if dead:
                        n_drop += 1
                    else:
                        keep.append(w)
                if len(keep) != len(si.on_wait):
                    si.on_wait = keep
            for u in si.on_update or []:
                if u.sync_type == "semaphore":
                    cum[(eng, u.id)] += u.update_value or 0
    return n_drop


def _get_nc():
    global _NC_CACHE
    if _NC_CACHE is None:
        nc = _build_kernel()
        _strip_dead_same_engine_waits(nc)
        _NC_CACHE = nc
    return _NC_CACHE


def _run(inputs, trace=False):
    y = np.asarray(inputs["y"])
    W1 = np.asarray(inputs["W1"], dtype=np.float32)
    W2 = np.asarray(inputs["W2"], dtype=np.float32)
    bias2 = np.asarray(inputs["bias2"], dtype=np.float32)

    # yT[p, t] = y.flat[t*128 + p]  (so column t is the per-partition scalar
    # vector for row-tile t of the flattened (b, i) dim)
    yT = np.ascontiguousarray(y.astype(np.float32).reshape(T, 128).T)

    in_maps = []
    for c in range(NCORES):
        in_maps.append(
            {
                "w1s": np.ascontiguousarray(W1[c * HS : (c + 1) * HS, :]),
                "w2s": np.ascontiguousarray(W2[:, c * HS : (c + 1) * HS]),
                "yt": yT,
            }
        )

    nc = _get_nc()
    bres = run_bass_kernel_spmd(nc, in_maps, core_ids=list(range(NCORES)), trace=trace)
    res = bres.results

    pert = np.empty((B, O, H), dtype=np.float32)
    s_total = np.zeros(O, dtype=np.float32)
    for c in range(NCORES):
        pert[:, :, c * HS : (c + 1) * HS] = res[c]["pert_out"].reshape(B, O, HS)
        # s_out[p, h] = sum_j W2s[h*128+p, j]  ->  index i = h*128 + p
        s_total += res[c]["s_out"].T.reshape(O)

    nn_row = (np.float32(H) * s_total + bias2).astype(np.float32)
    nn_output = np.broadcast_to(nn_row, (B, O)).copy()
    return (nn_output, pert), bres


def kernel(**inputs):
    (nn_output, pert), _ = _run(inputs, trace=False)
    return nn_output, pert
